# revision 11
# baseline (speedup 1.0000x reference)
"""Trainium2 Bass kernel for CausalSelfAttention (B=4, T=2048, C=768, H=6, D=128)
with RoPE + QK-RMSNorm.

Sharding: 8 cores = batch(4) x head-group(2, 3 heads each). Each core:
  - computes Q^T,K^T in (D, T) layout and V in (T, D) layout for its 3 heads
  - RoPE + RMSNorm on Q/K (partition-dim reductions via ones-matmul on PE)
  - causal attention with scores computed transposed (S^T: T_k on partitions,
    T_q on free dim) so softmax denom + AV matmuls need no transposes
  - partial c_proj over its 384 input channels
Host sums the two head-group partials per batch.
"""

import numpy as np

_B, _T, _C, _H, _D = 4, 2048, 768, 6, 128
_HPG = 3            # heads per group
_HD = _HPG * _D     # 384, per-group head dims
_NT = 4             # T tiles of 512
_TW = 512           # tile width (T_q)
_NKC = _T // 128    # 16 k-chunks of 128
_NCB = _C // 128    # 6 c_in chunks
_EPS = 1e-15

_cached = {}


def _build_nc():
    from contextlib import ExitStack
    from concourse import bacc, tile, mybir

    f32 = mybir.dt.float32
    f32r = mybir.dt.float32r
    Act = mybir.ActivationFunctionType
    Op = mybir.AluOpType

    nc = bacc.Bacc("TRN2", target_bir_lowering=False, debug=False)

    xT = nc.dram_tensor("xT", (_C, _T), f32r, kind="ExternalInput").ap()
    wq = nc.dram_tensor("wq", (_C, _HD), f32r, kind="ExternalInput").ap()
    wk = nc.dram_tensor("wk", (_C, _HD), f32r, kind="ExternalInput").ap()
    wv = nc.dram_tensor("wv", (_C, _HD), f32r, kind="ExternalInput").ap()
    wo = nc.dram_tensor("wo", (_HD, _C), f32r, kind="ExternalInput").ap()
    cc = nc.dram_tensor("cc", (128, _T), f32r, kind="ExternalInput").ap()
    ss = nc.dram_tensor("ss", (128, _T), f32r, kind="ExternalInput").ap()
    tri = nc.dram_tensor("tri", (128, 128), f32r, kind="ExternalInput").ap()
    ones = nc.dram_tensor("ones", (128, 128), f32r, kind="ExternalInput").ap()
    out = nc.dram_tensor("out", (_T, _C), f32, kind="ExternalOutput").ap()

    with tile.TileContext(nc) as tc, ExitStack() as ctx, \
            nc.allow_low_precision(reason="f32r tiles carry full fp32 bits; PE rounds at ingest"):
        # --- pools ---
        pc = ctx.enter_context(tc.tile_pool(name="pc", bufs=1))
        pg = ctx.enter_context(tc.tile_pool(name="pg", bufs=2))         # Q tile scratch
        pa = ctx.enter_context(tc.tile_pool(name="pa", bufs=4))         # A chunks
        pz = ctx.enter_context(tc.tile_pool(name="pz", bufs=6))         # Z chunks
        psm = ctx.enter_context(tc.tile_pool(name="psm", bufs=2))       # small (1,512)/(128,512)
        pob = ctx.enter_context(tc.tile_pool(name="pob", bufs=2))       # out staging
        # psum pools (8 banks total)
        ppq = ctx.enter_context(tc.tile_pool(name="ppq", bufs=1, space="PSUM"))
        pps = ctx.enter_context(tc.tile_pool(name="pps", bufs=2, space="PSUM"))
        ppo = ctx.enter_context(tc.tile_pool(name="ppo", bufs=2, space="PSUM"))
        ppm = ctx.enter_context(tc.tile_pool(name="ppm", bufs=1, space="PSUM"))
        ppb = ctx.enter_context(tc.tile_pool(name="ppb", bufs=2, space="PSUM"))

        # --- constants / inputs resident in SBUF ---
        t_cc = pc.tile([128, _T], f32r, tag="cc")
        t_ss = pc.tile([128, _T], f32r, tag="ss")
        t_tri = pc.tile([128, 128], f32r, tag="tri")
        nc.sync.dma_start(t_cc[:], cc[:])
        nc.sync.dma_start(t_ss[:], ss[:])
        nc.sync.dma_start(t_tri[:], tri[:])

        t_xt = []
        for c in range(_NCB):
            t = pc.tile([128, _T], f32r, tag=f"xt{c}")
            nc.sync.dma_start(t[:], xT[c * 128:(c + 1) * 128, :])
            t_xt.append(t)

        t_wq, t_wk, t_wv = [], [], []
        for name, dram, lst in (("wq", wq, t_wq), ("wk", wk, t_wk), ("wv", wv, t_wv)):
            for c in range(_NCB):
                t = pc.tile([128, _HD], f32r, tag=f"{name}{c}")
                nc.sync.dma_start(t[:], dram[c * 128:(c + 1) * 128, :])
                lst.append(t)
        t_wo = []
        for c in range(_HPG):
            t = pc.tile([128, _C], f32r, tag=f"wo{c}")
            nc.sync.dma_start(t[:], wo[c * 128:(c + 1) * 128, :])
            t_wo.append(t)

        t_ones = pc.tile([128, 128], f32r, tag="ones")
        nc.sync.dma_start(t_ones[:], ones[:])
        t_ones_col = t_ones[:, 0:1]
        t_ones_row = t_ones[0:1, :]
        t_eps = pc.tile([128, 1], f32, tag="eps")
        nc.gpsimd.memset(t_eps[:], _EPS)

        # persistent K^T (post rope+norm) per head, and V blocks
        t_kn = [pc.tile([128, _T], f32r, tag=f"kn{h}", name=f"kn{h}") for h in range(_HPG)]
        t_v = [pc.tile([128, _HD], f32r, tag=f"v{tb}", name=f"v{tb}") for tb in range(_NKC)]

        def rope_norm(dst_ap, tw, col0):
            """In-place RoPE + RMSNorm on dst_ap (128, tw) holding raw head
            projection in (D, T) layout, T-columns [col0, col0+tw)."""
            csl = slice(col0, col0 + tw)
            # half-swap copy (no negation: ss lower half is pre-negated on host)
            t_sw = pg.tile([128, _TW], f32r, tag="sw512", name="sw512")
            sw = t_sw[:, :tw]
            nc.sync.dma_start(sw[0:64, :], dst_ap[64:128, :])
            nc.sync.dma_start(sw[64:128, :], dst_ap[0:64, :])
            nc.vector.tensor_mul(dst_ap, dst_ap, t_cc[:, csl])
            nc.vector.tensor_mul(sw, sw, t_ss[:, csl])
            nc.vector.tensor_add(dst_ap, dst_ap, sw)
            # rmsnorm: ms = colsum(y^2)/128 via ones-matmul, then bcast+rsqrt
            t_sq = pg.tile([128, _TW], f32r, tag="sq512", name="sq512")
            sq = t_sq[:, :tw]
            nc.vector.tensor_mul(sq, dst_ap, dst_ap)
            for i in range(tw // _TW):
                isl = slice(i * _TW, (i + 1) * _TW)
                p_ms = ppm.tile([1, _TW], f32, tag="pms")
                nc.tensor.matmul(p_ms[:], t_ones_col, sq[:, isl],
                                 start=True, stop=True)
                t_ms = psm.tile([1, _TW], f32r, tag="ms")
                nc.scalar.copy(t_ms[:], p_ms[:])
                p_bc = ppb.tile([128, _TW], f32, tag="pbc")
                nc.tensor.matmul(p_bc[:], t_ones_row, t_ms[:],
                                 start=True, stop=True)
                t_sd = psm.tile([128, _TW], f32r, tag="sd")
                nc.scalar.activation(t_sd[:], p_bc[:], Act.Sqrt,
                                     bias=t_eps[:], scale=1.0 / 128.0)
                nc.vector.reciprocal(t_sd[:], t_sd[:])
                dsl = dst_ap[:, isl]
                nc.vector.tensor_mul(dsl, dsl, t_sd[:])

        # ---------------- Phase A: K^T (rope+norm) and V ----------------
        for h in range(_HPG):
            hsl = slice(h * 128, (h + 1) * 128)
            for i in range(_NT):
                isl = slice(i * _TW, (i + 1) * _TW)
                p_k = pps.tile([128, _TW], f32, tag="ps")
                for c in range(_NCB):
                    nc.tensor.matmul(p_k[:], t_wk[c][:, hsl], t_xt[c][:, isl],
                                     start=(c == 0), stop=(c == _NCB - 1))
                nc.vector.tensor_copy(t_kn[h][:, isl], p_k[:])
            for i in range(_NT):
                rope_norm(t_kn[h][:, i * _TW:(i + 1) * _TW], _TW, i * _TW)

        for tb in range(_NKC):
            bsl = slice(tb * 128, (tb + 1) * 128)
            p_v = ppo.tile([128, _HD], f32, tag="po")
            for c in range(_NCB):
                nc.tensor.matmul(p_v[:], t_xt[c][:, bsl], t_wv[c][:],
                                 start=(c == 0), stop=(c == _NCB - 1))
            nc.scalar.copy(t_v[tb][:], p_v[:])

        # ---------------- Phase B: per T_q tile ----------------
        for qt in range(_NT):
            qsl = slice(qt * _TW, (qt + 1) * _TW)
            z_chunks = []
            for h in range(_HPG):
                hsl = slice(h * 128, (h + 1) * 128)
                # Q^T raw projection for this (head, tile)
                p_q = ppq.tile([128, _TW], f32, tag="pq")
                for c in range(_NCB):
                    nc.tensor.matmul(p_q[:], t_wq[c][:, hsl], t_xt[c][:, qsl],
                                     start=(c == 0), stop=(c == _NCB - 1))
                t_g = pg.tile([128, _TW], f32r, tag="g")
                nc.vector.tensor_copy(t_g[:], p_q[:])
                rope_norm(t_g[:], _TW, qt * _TW)

                # attention: S^T chunks, exp, mask, denom, AV
                nchunk = 4 * qt + 4
                p_den = ppm.tile([1, _TW], f32, tag="pms")
                p_o = ppo.tile([128, _TW], f32, tag="po")
                for kc in range(nchunk):
                    roff = 0 if kc < 4 * qt else (kc - 4 * qt) * 128
                    nsl = slice(roff, _TW)
                    ksl = slice(kc * 128, (kc + 1) * 128)
                    p_s = pps.tile([128, _TW], f32, tag="ps")
                    nc.tensor.matmul(p_s[:, nsl], t_kn[h][:, ksl], t_g[:, nsl],
                                     start=True, stop=True)
                    t_a = pa.tile([128, _TW], f32r, tag="a")
                    nc.scalar.activation(t_a[:, nsl], p_s[:, nsl], Act.Exp,
                                         scale=1.0 / float(np.sqrt(_D)))
                    if kc >= 4 * qt:  # diagonal chunk: triangular mask
                        dsl = slice(roff, roff + 128)
                        nc.vector.tensor_mul(t_a[:, dsl], t_a[:, dsl], t_tri[:])
                    nc.tensor.matmul(p_den[:, nsl], t_ones_col, t_a[:, nsl],
                                     start=(kc == 0), stop=(kc == nchunk - 1))
                    nc.tensor.matmul(p_o[:, nsl], t_v[kc][:, hsl], t_a[:, nsl],
                                     start=(kc == 0), stop=(kc == nchunk - 1))
                # normalize: Z = O_unnorm * (1/den) broadcast
                t_den = psm.tile([1, _TW], f32r, tag="ms")
                nc.scalar.copy(t_den[:], p_den[:])
                p_db = ppb.tile([128, _TW], f32, tag="pbc")
                nc.tensor.matmul(p_db[:], t_ones_row, t_den[:],
                                 start=True, stop=True)
                t_rc2 = psm.tile([128, _TW], f32r, tag="sd", name="t_rc2")
                nc.vector.reciprocal(t_rc2[:], p_db[:])
                t_z = pz.tile([128, _TW], f32r, tag="z")
                nc.vector.tensor_mul(t_z[:], p_o[:], t_rc2[:])
                z_chunks.append(t_z)

            # c_proj for this tile: partial out rows [qt*512, qt*512+512)
            for tb in range(4):
                bsl = slice(tb * 128, (tb + 1) * 128)
                for nh in range(2):
                    osl = slice(nh * 384, (nh + 1) * 384)
                    p_c = ppq.tile([128, 384], f32, tag="pq")
                    for c in range(_HPG):
                        nc.tensor.matmul(p_c[:], z_chunks[c][:, bsl],
                                         t_wo[c][:, osl],
                                         start=(c == 0), stop=(c == _HPG - 1))
                    t_ob = pob.tile([128, 384], f32, tag="ob")
                    nc.scalar.copy(t_ob[:], p_c[:])
                    nc.sync.dma_start(
                        out[qt * _TW + tb * 128: qt * _TW + (tb + 1) * 128, osl],
                        t_ob[:])

    nc.compile()
    return nc


def _get_nc():
    if "nc" not in _cached:
        _cached["nc"] = _build_nc()
    return _cached["nc"]


def kernel(x, cos, sin, Wq, Wk, Wv, Wo):
    from concourse.bass_utils import run_bass_kernel_spmd

    x = np.asarray(x, dtype=np.float32)
    cos = np.asarray(cos, dtype=np.float32)
    sin = np.asarray(sin, dtype=np.float32)
    Wq = np.asarray(Wq, dtype=np.float32)
    Wk = np.asarray(Wk, dtype=np.float32)
    Wv = np.asarray(Wv, dtype=np.float32)
    Wo = np.asarray(Wo, dtype=np.float32)

    nc = _get_nc()

    cosT = np.ascontiguousarray(cos.reshape(_T, _D // 2).T)  # (64, T)
    sinT = np.ascontiguousarray(sin.reshape(_T, _D // 2).T)
    cc = np.concatenate([cosT, cosT], axis=0)                # (128, T)
    ss = np.concatenate([sinT, -sinT], axis=0)
    tri = (np.arange(128)[None, :] >= np.arange(128)[:, None]).astype(np.float32)
    ones128 = np.ones((128, 128), dtype=np.float32)

    in_maps = []
    for core in range(8):
        b, g = divmod(core, 2)
        gsl = slice(g * _HD, (g + 1) * _HD)
        in_maps.append({
            "xT": np.ascontiguousarray(x[b].T),
            "wq": np.ascontiguousarray(Wq[gsl, :].T),
            "wk": np.ascontiguousarray(Wk[gsl, :].T),
            "wv": np.ascontiguousarray(Wv[gsl, :].T),
            "wo": np.ascontiguousarray(Wo[:, gsl].T),
            "cc": cc, "ss": ss, "tri": tri, "ones": ones128,
        })

    res = run_bass_kernel_spmd(nc, in_maps, core_ids=list(range(8)))
    outs = [r_["out"] for r_ in res.results]
    return np.stack([outs[2 * b] + outs[2 * b + 1] for b in range(_B)], axis=0)


# revision 25
# speedup vs baseline: 1.1583x; 1.1583x over previous
"""Trainium2 Bass kernel for CausalSelfAttention (B=4, T=2048, C=768, H=6, D=128)
with RoPE + QK-RMSNorm.

Sharding: 8 cores = batch(4) x head-group(2, 3 heads each). Each core:
  - computes Q^T,K^T in (D, T) layout and V in (T, D) layout for its 3 heads
  - RoPE + RMSNorm on Q/K (partition-dim reductions via ones-matmul on PE)
  - causal attention with scores computed transposed (S^T: T_k on partitions,
    T_q on free dim) so softmax denom + AV matmuls need no transposes
  - partial c_proj over its 384 input channels
Host sums the two head-group partials per batch.
"""

import numpy as np

_B, _T, _C, _H, _D = 4, 2048, 768, 6, 128
_HPG = 3            # heads per group
_HD = _HPG * _D     # 384, per-group head dims
_NT = 4             # T tiles of 512
_TW = 512           # tile width (T_q)
_NKC = _T // 128    # 16 k-chunks of 128
_NCB = _C // 128    # 6 c_in chunks
_EPS = 1e-15

_cached = {}


def _build_nc():
    from contextlib import ExitStack
    from concourse import bacc, tile, mybir

    f32 = mybir.dt.float32
    f32r = mybir.dt.float32r
    Act = mybir.ActivationFunctionType
    Op = mybir.AluOpType

    nc = bacc.Bacc("TRN2", target_bir_lowering=False, debug=False)

    xT = nc.dram_tensor("xT", (_C, _T), f32r, kind="ExternalInput").ap()
    wq = nc.dram_tensor("wq", (_C, _HD), f32r, kind="ExternalInput").ap()
    wk = nc.dram_tensor("wk", (_C, _HD), f32r, kind="ExternalInput").ap()
    wv = nc.dram_tensor("wv", (_C, _HD), f32r, kind="ExternalInput").ap()
    wo = nc.dram_tensor("wo", (_HD, _C), f32r, kind="ExternalInput").ap()
    cc = nc.dram_tensor("cc", (128, _T), f32r, kind="ExternalInput").ap()
    ss = nc.dram_tensor("ss", (128, _T), f32r, kind="ExternalInput").ap()
    tri = nc.dram_tensor("tri", (128, 128), f32r, kind="ExternalInput").ap()
    ones = nc.dram_tensor("ones", (128, 128), f32r, kind="ExternalInput").ap()
    perm = nc.dram_tensor("perm", (128, 128), f32r, kind="ExternalInput").ap()
    out = nc.dram_tensor("out", (_T, _C), f32, kind="ExternalOutput").ap()

    with tile.TileContext(nc) as tc, ExitStack() as ctx, \
            nc.allow_low_precision(reason="f32r tiles carry full fp32 bits; PE rounds at ingest"):
        # --- pools ---
        pc = ctx.enter_context(tc.tile_pool(name="pc", bufs=1))
        pg = ctx.enter_context(tc.tile_pool(name="pg", bufs=2))         # Q tile scratch
        pa = ctx.enter_context(tc.tile_pool(name="pa", bufs=4))         # A chunks
        pz = ctx.enter_context(tc.tile_pool(name="pz", bufs=6))         # Z chunks
        psm = ctx.enter_context(tc.tile_pool(name="psm", bufs=2))       # small (1,512)/(128,512)
        pob = ctx.enter_context(tc.tile_pool(name="pob", bufs=2))       # out staging
        # psum pools (8 banks total)
        ppq = ctx.enter_context(tc.tile_pool(name="ppq", bufs=2, space="PSUM"))
        pps = ctx.enter_context(tc.tile_pool(name="pps", bufs=2, space="PSUM"))
        ppo = ctx.enter_context(tc.tile_pool(name="ppo", bufs=2, space="PSUM"))
        ppm = ctx.enter_context(tc.tile_pool(name="ppm", bufs=1, space="PSUM"))
        ppd = ctx.enter_context(tc.tile_pool(name="ppd", bufs=1, space="PSUM"))

        # --- constants / inputs resident in SBUF ---
        t_cc = pc.tile([128, _T], f32r, tag="cc")
        t_ss = pc.tile([128, _T], f32r, tag="ss")
        t_tri = pc.tile([128, 128], f32r, tag="tri")
        nc.sync.dma_start(t_cc[:], cc[:])
        nc.sync.dma_start(t_ss[:], ss[:])
        nc.sync.dma_start(t_tri[:], tri[:])

        t_xt = []
        for c in range(_NCB):
            t = pc.tile([128, _T], f32r, tag=f"xt{c}")
            nc.sync.dma_start(t[:], xT[c * 128:(c + 1) * 128, :])
            t_xt.append(t)

        t_wq, t_wk, t_wv = [], [], []
        for name, dram, lst in (("wq", wq, t_wq), ("wk", wk, t_wk), ("wv", wv, t_wv)):
            for c in range(_NCB):
                t = pc.tile([128, _HD], f32r, tag=f"{name}{c}")
                nc.sync.dma_start(t[:], dram[c * 128:(c + 1) * 128, :])
                lst.append(t)
        t_wo = []
        for c in range(_HPG):
            t = pc.tile([128, _C], f32r, tag=f"wo{c}")
            nc.sync.dma_start(t[:], wo[c * 128:(c + 1) * 128, :])
            t_wo.append(t)

        t_ones = pc.tile([128, 128], f32r, tag="ones")
        t_perm = pc.tile([128, 128], f32r, tag="perm")
        nc.sync.dma_start(t_ones[:], ones[:])
        nc.sync.dma_start(t_perm[:], perm[:])
        t_ones_col = t_ones[:, 0:1]
        t_ones_row = t_ones[0:1, :]
        t_eps = pc.tile([128, 1], f32, tag="eps")
        nc.gpsimd.memset(t_eps[:], _EPS)

        # persistent K^T (post rope+norm) per head, and V blocks
        t_kn = [pc.tile([128, _T], f32r, tag=f"kn{h}", name=f"kn{h}") for h in range(_HPG)]
        t_v = [pc.tile([128, _HD], f32r, tag=f"v{tb}", name=f"v{tb}") for tb in range(_NKC)]

        def rope_norm(dst_ap, tw, col0):
            """In-place RoPE + RMSNorm on dst_ap (128, tw) holding raw head
            projection in (D, T) layout, T-columns [col0, col0+tw)."""
            csl = slice(col0, col0 + tw)
            # half-swap copy (no negation: ss lower half is pre-negated on host)
            p_sw = pps.tile([128, _TW], f32, tag="ps", name="p_sw")
            nc.tensor.matmul(p_sw[:], t_perm[:], dst_ap, start=True, stop=True)
            t_sw = pg.tile([128, _TW], f32r, tag="sw512", name="sw512")
            sw = t_sw[:, :tw]
            nc.vector.tensor_mul(dst_ap, dst_ap, t_cc[:, csl])
            nc.vector.tensor_mul(sw, p_sw[:], t_ss[:, csl])
            nc.vector.tensor_add(dst_ap, dst_ap, sw)
            # rmsnorm: ms = colsum(y^2)/128 via ones-matmul, then bcast+rsqrt
            t_sq = pg.tile([128, _TW], f32r, tag="sq512", name="sq512")
            sq = t_sq[:, :tw]
            nc.vector.tensor_mul(sq, dst_ap, dst_ap)
            for i in range(tw // _TW):
                isl = slice(i * _TW, (i + 1) * _TW)
                p_ms = ppm.tile([1, _TW], f32, tag="pms")
                nc.tensor.matmul(p_ms[:], t_ones_col, sq[:, isl],
                                 start=True, stop=True)
                t_ms = psm.tile([1, _TW], f32r, tag="ms")
                nc.scalar.copy(t_ms[:], p_ms[:])
                p_bc = pps.tile([128, _TW], f32, tag="ps", name="p_bc")
                nc.tensor.matmul(p_bc[:], t_ones_row, t_ms[:],
                                 start=True, stop=True)
                t_sd = psm.tile([128, _TW], f32r, tag="sd")
                nc.scalar.activation(t_sd[:], p_bc[:], Act.Sqrt,
                                     bias=t_eps[:], scale=1.0 / 128.0)
                nc.vector.reciprocal(t_sd[:], t_sd[:])
                dsl = dst_ap[:, isl]
                nc.vector.tensor_mul(dsl, dsl, t_sd[:])

        # ---------------- Phase A: K^T (rope+norm) and V ----------------
        for h in range(_HPG):
            hsl = slice(h * 128, (h + 1) * 128)
            for i in range(_NT):
                isl = slice(i * _TW, (i + 1) * _TW)
                p_k = pps.tile([128, _TW], f32, tag="ps")
                for c in range(_NCB):
                    nc.tensor.matmul(p_k[:], t_wk[c][:, hsl], t_xt[c][:, isl],
                                     start=(c == 0), stop=(c == _NCB - 1))
                nc.vector.tensor_copy(t_kn[h][:, isl], p_k[:])
        for h in range(_HPG):
            for i in range(_NT):
                rope_norm(t_kn[h][:, i * _TW:(i + 1) * _TW], _TW, i * _TW)

        for tb in range(_NKC):
            bsl = slice(tb * 128, (tb + 1) * 128)
            p_v = ppo.tile([128, _HD], f32, tag="po")
            for c in range(_NCB):
                nc.tensor.matmul(p_v[:], t_xt[c][:, bsl], t_wv[c][:],
                                 start=(c == 0), stop=(c == _NCB - 1))
            nc.scalar.copy(t_v[tb][:], p_v[:])

        # ---------------- Phase B: per T_q tile ----------------
        a_ctr = [0]

        def q_chain(qt, h):
            qsl = slice(qt * _TW, (qt + 1) * _TW)
            hsl = slice(h * 128, (h + 1) * 128)
            p_q = ppq.tile([128, _TW], f32, tag="pq", name="p_q")
            for c in range(_NCB):
                nc.tensor.matmul(p_q[:], t_wq[c][:, hsl], t_xt[c][:, qsl],
                                 start=(c == 0), stop=(c == _NCB - 1))
            t_g = pg.tile([128, _TW], f32r, tag="g", name="g", bufs=4)
            nc.vector.tensor_copy(t_g[:], p_q[:])
            rope_norm(t_g[:], _TW, qt * _TW)
            return t_g

        def attention(qt, h, t_g):
            hsl = slice(h * 128, (h + 1) * 128)
            nchunk = 4 * qt + 4
            p_den = ppd.tile([1, _TW], f32, tag="pd", name="p_den")
            p_o = ppo.tile([128, _TW], f32, tag="po", name="p_o")
            for kc in range(nchunk):
                roff = 0 if kc < 4 * qt else (kc - 4 * qt) * 128
                nsl = slice(roff, _TW)
                ksl = slice(kc * 128, (kc + 1) * 128)
                p_s = pps.tile([128, _TW], f32, tag="ps", name="p_s")
                nc.tensor.matmul(p_s[:, nsl], t_kn[h][:, ksl], t_g[:, nsl],
                                 start=True, stop=True)
                t_a = pc.tile([128, _TW], f32r, tag=f"wk{a_ctr[0] % _NCB}",
                              name=f"a{a_ctr[0] % _NCB}")
                a_ctr[0] += 1
                nc.scalar.activation(t_a[:, nsl], p_s[:, nsl], Act.Exp,
                                     scale=1.0 / float(np.sqrt(_D)))
                if kc >= 4 * qt:  # diagonal chunk: triangular mask
                    dsl = slice(roff, roff + 128)
                    nc.vector.tensor_mul(t_a[:, dsl], t_a[:, dsl], t_tri[:])
                nc.tensor.matmul(p_den[:, nsl], t_ones_col, t_a[:, nsl],
                                 start=(kc == 0), stop=(kc == nchunk - 1))
                nc.tensor.matmul(p_o[:, nsl], t_v[kc][:, hsl], t_a[:, nsl],
                                 start=(kc == 0), stop=(kc == nchunk - 1))
            t_den = psm.tile([1, _TW], f32r, tag="ms")
            nc.scalar.copy(t_den[:], p_den[:])
            p_db = pps.tile([128, _TW], f32, tag="ps", name="p_db")
            nc.tensor.matmul(p_db[:], t_ones_row, t_den[:],
                             start=True, stop=True)
            t_rc2 = psm.tile([128, _TW], f32r, tag="sd", name="t_rc2")
            nc.vector.reciprocal(t_rc2[:], p_db[:])
            zi = h + _HPG * (qt % 2)
            t_z = pc.tile([128, _TW], f32r, tag=f"wv{zi}", name=f"z{zi}")
            nc.vector.tensor_mul(t_z[:], p_o[:], t_rc2[:])
            return t_z

        for qt in range(_NT):
            gs = [q_chain(qt, h) for h in range(_HPG)]
            z_chunks = [attention(qt, h, gs[h]) for h in range(_HPG)]
            # c_proj for this tile: partial out rows [qt*512, qt*512+512)
            for tb in range(4):
                bsl = slice(tb * 128, (tb + 1) * 128)
                t_ob = pob.tile([128, _C], f32, tag="ob")
                for nh in range(2):
                    osl = slice(nh * 384, (nh + 1) * 384)
                    p_c = ppq.tile([128, 384], f32, tag="pq")
                    for c in range(_HPG):
                        nc.tensor.matmul(p_c[:], z_chunks[c][:, bsl],
                                         t_wo[c][:, osl],
                                         start=(c == 0), stop=(c == _HPG - 1))
                    nc.scalar.copy(t_ob[:, osl], p_c[:])
                nc.sync.dma_start(
                    out[qt * _TW + tb * 128: qt * _TW + (tb + 1) * 128, :],
                    t_ob[:])

    nc.compile()
    return nc


def _get_nc():
    if "nc" not in _cached:
        _cached["nc"] = _build_nc()
    return _cached["nc"]


def make_in_maps(x, cos, sin, Wq, Wk, Wv, Wo):
    cosT = np.ascontiguousarray(cos.reshape(_T, _D // 2).T)  # (64, T)
    sinT = np.ascontiguousarray(sin.reshape(_T, _D // 2).T)
    cc = np.concatenate([cosT, cosT], axis=0)                # (128, T)
    ss = np.concatenate([sinT, -sinT], axis=0)
    tri = (np.arange(128)[None, :] >= np.arange(128)[:, None]).astype(np.float32)
    ones128 = np.ones((128, 128), dtype=np.float32)
    permm = np.zeros((128, 128), dtype=np.float32)           # half-swap permutation
    for d in range(64):
        permm[64 + d, d] = 1.0
        permm[d, 64 + d] = 1.0
    in_maps = []
    for core in range(8):
        b, g = divmod(core, 2)
        gsl = slice(g * _HD, (g + 1) * _HD)
        in_maps.append({
            "xT": np.ascontiguousarray(x[b].T),
            "wq": np.ascontiguousarray(Wq[gsl, :].T),
            "wk": np.ascontiguousarray(Wk[gsl, :].T),
            "wv": np.ascontiguousarray(Wv[gsl, :].T),
            "wo": np.ascontiguousarray(Wo[:, gsl].T),
            "cc": cc, "ss": ss, "tri": tri, "ones": ones128, "perm": permm,
        })
    return in_maps


def kernel(x, cos, sin, Wq, Wk, Wv, Wo):
    from concourse.bass_utils import run_bass_kernel_spmd

    x = np.asarray(x, dtype=np.float32)
    cos = np.asarray(cos, dtype=np.float32)
    sin = np.asarray(sin, dtype=np.float32)
    Wq = np.asarray(Wq, dtype=np.float32)
    Wk = np.asarray(Wk, dtype=np.float32)
    Wv = np.asarray(Wv, dtype=np.float32)
    Wo = np.asarray(Wo, dtype=np.float32)

    nc = _get_nc()
    in_maps = make_in_maps(x, cos, sin, Wq, Wk, Wv, Wo)
    res = run_bass_kernel_spmd(nc, in_maps, core_ids=list(range(8)))
    outs = [r_["out"] for r_ in res.results]
    return np.stack([outs[2 * b] + outs[2 * b + 1] for b in range(_B)], axis=0)


# revision 34
# speedup vs baseline: 288.0042x; 248.6380x over previous
"""Trainium2 Bass kernel for CausalSelfAttention (B=4, T=2048, C=768, H=6, D=128)
with RoPE + QK-RMSNorm.

Sharding: 8 cores = batch(4) x head-group(2, 3 heads each). Each core:
  - computes Q^T,K^T in (D, T) layout and V in (T, D) layout for its 3 heads
  - RoPE + RMSNorm on Q/K (partition-dim reductions via ones-matmul on PE)
  - causal attention with scores computed transposed (S^T: T_k on partitions,
    T_q on free dim) so softmax denom + AV matmuls need no transposes
  - partial c_proj over its 384 input channels
Host sums the two head-group partials per batch.
"""

import numpy as np

_B, _T, _C, _H, _D = 4, 2048, 768, 6, 128
_HPG = 3            # heads per group
_HD = _HPG * _D     # 384, per-group head dims
_NT = 4             # T tiles of 512
_TW = 512           # tile width (T_q)
_NKC = _T // 128    # 16 k-chunks of 128
_NCB = _C // 128    # 6 c_in chunks
_EPS = 1e-15

_cached = {}


def _build_nc():
    from contextlib import ExitStack
    from concourse import bacc, tile, mybir

    f32 = mybir.dt.float32
    f32r = mybir.dt.float32r
    Act = mybir.ActivationFunctionType
    Op = mybir.AluOpType

    nc = bacc.Bacc("TRN2", target_bir_lowering=False, debug=False)

    xT = nc.dram_tensor("xT", (_C, _T), f32r, kind="ExternalInput").ap()
    wq = nc.dram_tensor("wq", (_C, _HD), f32r, kind="ExternalInput").ap()
    wk = nc.dram_tensor("wk", (_C, _HD), f32r, kind="ExternalInput").ap()
    wv = nc.dram_tensor("wv", (_C, _HD), f32r, kind="ExternalInput").ap()
    wo = nc.dram_tensor("wo", (_HD, _C), f32r, kind="ExternalInput").ap()
    cc = nc.dram_tensor("cc", (128, _T), f32r, kind="ExternalInput").ap()
    ss = nc.dram_tensor("ss", (128, _T), f32r, kind="ExternalInput").ap()
    tri = nc.dram_tensor("tri", (128, 128), f32r, kind="ExternalInput").ap()
    ones = nc.dram_tensor("ones", (128, 128), f32r, kind="ExternalInput").ap()
    perm = nc.dram_tensor("perm", (128, 128), f32r, kind="ExternalInput").ap()
    out = nc.dram_tensor("out", (_T, _C), f32, kind="ExternalOutput").ap()

    with tile.TileContext(nc) as tc, ExitStack() as ctx, \
            nc.allow_low_precision(reason="f32r tiles carry full fp32 bits; PE rounds at ingest"):
        # --- pools ---
        pc = ctx.enter_context(tc.tile_pool(name="pc", bufs=1))
        pg = ctx.enter_context(tc.tile_pool(name="pg", bufs=2))         # Q tile scratch
        pa = ctx.enter_context(tc.tile_pool(name="pa", bufs=4))         # A chunks
        pz = ctx.enter_context(tc.tile_pool(name="pz", bufs=6))         # Z chunks
        psm = ctx.enter_context(tc.tile_pool(name="psm", bufs=2))       # small (1,512)/(128,512)
        pob = ctx.enter_context(tc.tile_pool(name="pob", bufs=2))       # out staging
        # psum pools (8 banks total)
        ppq = ctx.enter_context(tc.tile_pool(name="ppq", bufs=2, space="PSUM"))
        pps = ctx.enter_context(tc.tile_pool(name="pps", bufs=2, space="PSUM"))
        ppo = ctx.enter_context(tc.tile_pool(name="ppo", bufs=2, space="PSUM"))
        ppd = ctx.enter_context(tc.tile_pool(name="ppd", bufs=1, space="PSUM"))
        ppm = ctx.enter_context(tc.tile_pool(name="ppm", bufs=1, space="PSUM"))

        # --- constants / inputs resident in SBUF ---
        # load order matters: wk + xT feed the first PE work (K-projections);
        # cc/ss are not needed until rope, wq not until phase B, wo until c_proj
        t_wq, t_wk, t_wv = [], [], []
        for c in range(_NCB):
            t = pc.tile([128, _HD], f32r, tag=f"wk{c}", name=f"wk{c}",
                        padded_shape=[128, _TW])
            nc.sync.dma_start(t[:], wk[c * 128:(c + 1) * 128, :])
            t_wk.append(t)
        t_xt = []
        for c in range(_NCB):
            t = pc.tile([128, _T], f32r, tag=f"xt{c}", name=f"xt{c}")
            nc.sync.dma_start(t[:], xT[c * 128:(c + 1) * 128, :])
            t_xt.append(t)
        for c in range(_NCB):
            t = pc.tile([128, _HD], f32r, tag=f"wv{c}", name=f"wv{c}",
                        padded_shape=[128, _TW])
            nc.sync.dma_start(t[:], wv[c * 128:(c + 1) * 128, :])
            t_wv.append(t)
        t_cc = pc.tile([128, _T], f32r, tag="cc")
        t_ss = pc.tile([128, _T], f32r, tag="ss")
        nc.sync.dma_start(t_cc[:], cc[:])
        nc.sync.dma_start(t_ss[:], ss[:])
        for c in range(_NCB):
            t = pc.tile([128, _HD], f32r, tag=f"wq{c}", name=f"wq{c}")
            nc.sync.dma_start(t[:], wq[c * 128:(c + 1) * 128, :])
            t_wq.append(t)
        t_tri = pc.tile([128, 128], f32r, tag="tri")
        t_ones = pc.tile([128, 128], f32r, tag="ones")
        t_perm = pc.tile([128, 128], f32r, tag="perm")
        nc.sync.dma_start(t_tri[:], tri[:])
        nc.sync.dma_start(t_ones[:], ones[:])
        nc.sync.dma_start(t_perm[:], perm[:])
        t_ones_col = t_ones[:, 0:1]
        t_ones_row = t_ones[0:1, :]
        t_eps = pc.tile([128, 1], f32, tag="eps")
        nc.gpsimd.memset(t_eps[:], _EPS)
        t_wo = []
        for c in range(_HPG):
            t = pc.tile([128, _C], f32r, tag=f"wo{c}", name=f"wo{c}")
            nc.sync.dma_start(t[:], wo[c * 128:(c + 1) * 128, :])
            t_wo.append(t)

        # persistent K^T (post rope+norm) per head, and V blocks
        t_kn = [pc.tile([128, _T], f32r, tag=f"kn{h}", name=f"kn{h}") for h in range(_HPG)]
        t_v = [pc.tile([128, _HD], f32r, tag=f"v{tb}", name=f"v{tb}") for tb in range(_NKC)]

        def rope_part(dst_ap, col0):
            """In-place RoPE on dst_ap (128, 512)."""
            csl = slice(col0, col0 + _TW)
            p_sw = pps.tile([128, _TW], f32, tag="ps", name="p_sw")
            nc.tensor.matmul(p_sw[:], t_perm[:], dst_ap, start=True, stop=True)
            t_sw = pg.tile([128, _TW], f32r, tag="sw512", name="sw512", bufs=3)
            nc.vector.tensor_mul(dst_ap, dst_ap, t_cc[:, csl])
            nc.vector.tensor_mul(t_sw[:], p_sw[:], t_ss[:, csl])
            nc.vector.tensor_add(dst_ap, dst_ap, t_sw[:])

        def norm_pre(dst_ap, bc_pool, bc_tag, ms_on_act=True):
            """Square + partition-sum + broadcast; returns bcast psum."""
            t_sq = pg.tile([128, _TW], f32r, tag="sq512", name="sq512", bufs=3)
            nc.vector.tensor_mul(t_sq[:], dst_ap, dst_ap)
            p_ms = ppm.tile([1, _TW], f32, tag="pms", name="p_ms")
            nc.tensor.matmul(p_ms[:], t_ones_col, t_sq[:], start=True, stop=True)
            t_ms = psm.tile([1, _TW], f32r, tag="ms", name="t_ms", bufs=3)
            if ms_on_act:
                nc.scalar.copy(t_ms[:], p_ms[:])
            else:
                nc.vector.tensor_copy(t_ms[:], p_ms[:])
            p_bc = bc_pool.tile([128, _TW], f32, tag=bc_tag, name="p_bc")
            nc.tensor.matmul(p_bc[:], t_ones_row, t_ms[:], start=True, stop=True)
            return p_bc

        def norm_post(dst_ap, p_bc):
            """sqrt -> reciprocal -> scale, in place on dst_ap."""
            t_sd = psm.tile([128, _TW], f32r, tag="sd", name="t_sd", bufs=3)
            nc.scalar.activation(t_sd[:], p_bc[:], Act.Sqrt,
                                 bias=t_eps[:], scale=1.0 / 128.0)
            nc.vector.reciprocal(t_sd[:], t_sd[:])
            nc.vector.tensor_mul(dst_ap, dst_ap, t_sd[:])

        # one bcast-psum route per head so three chains can be in flight
        _bc_routes = [(pps, "ps"), (ppq, "pq"), (ppd, "pd")]

        def rope_norm(dst_ap, tw, col0):
            rope_part(dst_ap, col0)
            norm_post(dst_ap, norm_pre(dst_ap, pps, "ps"))

        # ---------------- Phase A: K^T (rope+norm) and V ----------------
        for i in range(_NT):
            isl = slice(i * _TW, (i + 1) * _TW)
            for h in range(_HPG):
                hsl = slice(h * 128, (h + 1) * 128)
                p_k = pps.tile([128, _TW], f32, tag="ps")
                for c in range(_NCB):
                    nc.tensor.matmul(p_k[:], t_wk[c][:, hsl], t_xt[c][:, isl],
                                     start=(c == 0), stop=(c == _NCB - 1))
                nc.scalar.copy(t_kn[h][:, isl], p_k[:])
        # V-projs emitted here: independent PE work that fills the gaps in
        # the serial rope+norm chains below
        for tb in range(_NKC):
            bsl = slice(tb * 128, (tb + 1) * 128)
            p_v = ppo.tile([128, _HD], f32, tag="po")
            for c in range(_NCB):
                nc.tensor.matmul(p_v[:], t_xt[c][:, bsl], t_wv[c][:],
                                 start=(c == 0), stop=(c == _NCB - 1))
            nc.scalar.copy(t_v[tb][:], p_v[:])
        # stage-batched across heads: three chains in flight, each using its
        # own bcast-psum pool (ppq/ppd are otherwise idle in phase A)
        for i in range(_NT):
            isl = slice(i * _TW, (i + 1) * _TW)
            for h in range(_HPG):
                rope_part(t_kn[h][:, isl], i * _TW)
            bcs = []
            for h in range(_HPG):
                pool, tag = _bc_routes[h]
                bcs.append(norm_pre(t_kn[h][:, isl], pool, tag))
            for h in range(_HPG):
                norm_post(t_kn[h][:, isl], bcs[h])

        # ---------------- Phase B: per T_q tile ----------------
        a_ctr = [0]

        def q_chain(qt, h):
            qsl = slice(qt * _TW, (qt + 1) * _TW)
            hsl = slice(h * 128, (h + 1) * 128)
            p_q = ppq.tile([128, _TW], f32, tag="pq", name="p_q")
            for c in range(_NCB):
                nc.tensor.matmul(p_q[:], t_wq[c][:, hsl], t_xt[c][:, qsl],
                                 start=(c == 0), stop=(c == _NCB - 1))
            t_g = pg.tile([128, _TW], f32r, tag="g", name="g", bufs=7)
            nc.vector.tensor_copy(t_g[:], p_q[:])
            rope_part(t_g[:], qt * _TW)
            pool, tag = _bc_routes[h] if h < 2 else (pps, "ps")
            norm_post(t_g[:], norm_pre(t_g[:], pool, tag, ms_on_act=False))
            return t_g

        def attention(qt, h, t_g):
            """Causal attention for one (T_q tile, head). The den/AV matmuls
            are emitted LOOKAHEAD chunks behind the S/exp pair: the PE stream
            is in-order, so den(kc) stalls on exp(kc) unless later S-matmuls
            are issued first."""
            hsl = slice(h * 128, (h + 1) * 128)
            nchunk = 4 * qt + 4
            LOOKAHEAD = 3
            p_den = ppd.tile([1, _TW], f32, tag="pd", name="p_den")
            p_o = ppo.tile([128, _TW], f32, tag="po", name="p_o")
            a_tiles = {}

            def emit_s(kc):
                roff = 0 if kc < 4 * qt else (kc - 4 * qt) * 128
                nsl = slice(roff, _TW)
                ksl = slice(kc * 128, (kc + 1) * 128)
                p_s = pps.tile([128, _TW], f32, tag="ps", name="p_s")
                nc.tensor.matmul(p_s[:, nsl], t_kn[h][:, ksl], t_g[:, nsl],
                                 start=True, stop=True)
                t_a = pc.tile([128, _TW], f32r, tag=f"wk{a_ctr[0] % _NCB}",
                              name=f"a{a_ctr[0] % _NCB}")
                a_ctr[0] += 1
                nc.scalar.activation(t_a[:, nsl], p_s[:, nsl], Act.Exp,
                                     scale=1.0 / float(np.sqrt(_D)))
                if kc >= 4 * qt:  # diagonal chunk: triangular mask
                    dsl = slice(roff, roff + 128)
                    nc.vector.tensor_mul(t_a[:, dsl], t_a[:, dsl], t_tri[:])
                a_tiles[kc] = t_a

            def emit_acc(kc):
                roff = 0 if kc < 4 * qt else (kc - 4 * qt) * 128
                nsl = slice(roff, _TW)
                t_a = a_tiles.pop(kc)
                nc.tensor.matmul(p_den[:, nsl], t_ones_col, t_a[:, nsl],
                                 start=(kc == 0), stop=(kc == nchunk - 1))
                nc.tensor.matmul(p_o[:, nsl], t_v[kc][:, hsl], t_a[:, nsl],
                                 start=(kc == 0), stop=(kc == nchunk - 1))

            for kc in range(nchunk + LOOKAHEAD):
                if kc < nchunk:
                    emit_s(kc)
                if kc >= LOOKAHEAD:
                    emit_acc(kc - LOOKAHEAD)
            # normalize: Z = O_unnorm * (1/den) broadcast
            t_den = psm.tile([1, _TW], f32r, tag="ms", name="t_den", bufs=3)
            nc.scalar.copy(t_den[:], p_den[:])
            p_db = pps.tile([128, _TW], f32, tag="ps", name="p_db")
            nc.tensor.matmul(p_db[:], t_ones_row, t_den[:], start=True, stop=True)
            t_rc2 = psm.tile([128, _TW], f32r, tag="sd", name="t_rc2", bufs=3)
            nc.vector.reciprocal(t_rc2[:], p_db[:])
            zi = h + _HPG * (qt % 2)
            t_z = pc.tile([128, _TW], f32r, tag=f"wv{zi}", name=f"z{zi}")
            nc.vector.tensor_mul(t_z[:], p_o[:], t_rc2[:])
            return t_z

        for qt in range(_NT):
            gs = [q_chain(qt, h) for h in range(_HPG)]
            z_chunks = [attention(qt, h, gs[h]) for h in range(_HPG)]
            # c_proj for this tile: partial out rows [qt*512, qt*512+512)
            for tb in range(4):
                bsl = slice(tb * 128, (tb + 1) * 128)
                t_ob = pob.tile([128, _C], f32, tag="ob")
                for nh in range(2):
                    osl = slice(nh * 384, (nh + 1) * 384)
                    p_c = ppq.tile([128, 384], f32, tag="pq")
                    for c in range(_HPG):
                        nc.tensor.matmul(p_c[:], z_chunks[c][:, bsl],
                                         t_wo[c][:, osl],
                                         start=(c == 0), stop=(c == _HPG - 1))
                    nc.vector.tensor_copy(t_ob[:, osl], p_c[:])
                nc.sync.dma_start(
                    out[qt * _TW + tb * 128: qt * _TW + (tb + 1) * 128, :],
                    t_ob[:])

    nc.compile()
    return nc


def _get_nc():
    if "nc" not in _cached:
        _cached["nc"] = _build_nc()
    return _cached["nc"]


def make_in_maps(x, cos, sin, Wq, Wk, Wv, Wo):
    cosT = np.ascontiguousarray(cos.reshape(_T, _D // 2).T)  # (64, T)
    sinT = np.ascontiguousarray(sin.reshape(_T, _D // 2).T)
    cc = np.concatenate([cosT, cosT], axis=0)                # (128, T)
    ss = np.concatenate([sinT, -sinT], axis=0)
    tri = (np.arange(128)[None, :] >= np.arange(128)[:, None]).astype(np.float32)
    ones128 = np.ones((128, 128), dtype=np.float32)
    permm = np.zeros((128, 128), dtype=np.float32)           # half-swap permutation
    for d in range(64):
        permm[64 + d, d] = 1.0
        permm[d, 64 + d] = 1.0
    in_maps = []
    for core in range(8):
        b, g = divmod(core, 2)
        gsl = slice(g * _HD, (g + 1) * _HD)
        in_maps.append({
            "xT": np.ascontiguousarray(x[b].T),
            "wq": np.ascontiguousarray(Wq[gsl, :].T),
            "wk": np.ascontiguousarray(Wk[gsl, :].T),
            "wv": np.ascontiguousarray(Wv[gsl, :].T),
            "wo": np.ascontiguousarray(Wo[:, gsl].T),
            "cc": cc, "ss": ss, "tri": tri, "ones": ones128, "perm": permm,
        })
    return in_maps


def kernel(x, cos, sin, Wq, Wk, Wv, Wo):
    from concourse.bass_utils import run_bass_kernel_spmd

    x = np.asarray(x, dtype=np.float32)
    cos = np.asarray(cos, dtype=np.float32)
    sin = np.asarray(sin, dtype=np.float32)
    Wq = np.asarray(Wq, dtype=np.float32)
    Wk = np.asarray(Wk, dtype=np.float32)
    Wv = np.asarray(Wv, dtype=np.float32)
    Wo = np.asarray(Wo, dtype=np.float32)

    nc = _get_nc()
    in_maps = make_in_maps(x, cos, sin, Wq, Wk, Wv, Wo)
    res = run_bass_kernel_spmd(nc, in_maps, core_ids=list(range(8)))
    outs = [r_["out"] for r_ in res.results]
    return np.stack([outs[2 * b] + outs[2 * b + 1] for b in range(_B)], axis=0)
